# revision 40
# baseline (speedup 1.0000x reference)
"""Trainium2 Bass kernel for nn_InteractionModule (GNN message passing).

v3 strategy (8 NeuronCores, SPMD, no collectives):
 - Nodes sharded 8 x 6250 by dst; edges assigned to the core owning dst.
 - Table phase: every core computes spE[n] = ssp(W_diff ssp(x_n) + b_diff)
   (f16) over all N nodes in 32 windows of 1664, activations batched into
   same-function runs (4 windows/group) to avoid act-table reloads; bf16
   matmuls; dma_start_transpose forms row tiles; table stored across FOUR
   quarter tensors so edge gathers for quarter k depend only on quarter k.
 - Edges binned by (src quarter, dst chunk), laid out bin-major; supertile
   emission is interleaved with the table build so gathers/gates/scatters
   overlap table production.  Aggregation accumulates per (quarter, chunk)
   run in PSUM and is added into z_sT (which phase 1b pre-fills with
   ssp(z_same)); out1 = z_sT after all runs.
 - Residual stack in transposed [f, node] layout over two column halves,
   batched activations, outputs via DMA transpose in block layout.
"""

import numpy as np

N, E, F, K, R = 50000, 600000, 128, 64, 3
NC_ = 8
NSH = N // NC_            # 6250 nodes per core
CHUNK = 256               # scatter window (one-hot width)
NCHUNK = (NSH + CHUNK - 1) // CHUNK   # 25
WINA = 1664               # table window (13 blocks of 128)
NWA = 32                  # windows over padded node range
NPAD = NWA * WINA         # 53248 padded nodes
GRPA = 8                  # table windows per act-batch group
QT = 4                    # src-quarter bins
QROWS = NPAD // QT        # 13312 table rows per quarter (8 windows)
NSHP = 6656               # padded shard width (13*512)
NBLK = 49                 # output blocks (49*128 = 6272 >= 6250)
LOG2 = float(np.log(2.0))

_cache = {}


def _prep(x, edge_index, edge_attr):
    """Host-side sharding.

    Edges are binned by (src quarter, dst chunk) and laid out bin-major so
    the gathers for bin b depend only on table quarter b — overlapping the
    edge phase with the table build.  Each (quarter, chunk) run accumulates
    in its own PSUM tile and is added into z_sT when the run closes.
    """
    src = np.asarray(edge_index[0], dtype=np.int64)
    dst = np.asarray(edge_index[1], dtype=np.int64)
    core = dst // NSH
    dstl = dst - core * NSH
    ea = np.asarray(edge_attr, dtype=np.float32)

    chunk = dstl // CHUNK
    qt = src // QROWS
    binid = qt * NCHUNK + chunk                     # run = (quarter, chunk)
    key = (core * QT * NCHUNK + binid) * (N + 1) + src
    order = np.argsort(key, kind="stable")
    counts = np.bincount(core * QT * NCHUNK + binid,
                         minlength=NC_ * QT * NCHUNK).reshape(NC_, QT * NCHUNK)
    stq = (counts + 127) // 128
    stq = stq.max(axis=0)                           # [QT*NCHUNK] core-uniform
    S = int(stq.sum())
    pad_tail = (-S) % 8
    S += pad_tail
    G = S // 4

    rb = np.zeros(QT * NCHUNK + 1, np.int64)
    np.cumsum(stq, out=rb[1:])

    cq = np.zeros(S, np.int64)
    qtq = np.zeros(S, np.int64)
    run_first = np.zeros(S, bool)
    run_last = np.zeros(S, bool)
    for r in range(QT * NCHUNK):
        if stq[r] == 0:
            continue
        cq[rb[r] : rb[r + 1]] = r % NCHUNK
        qtq[rb[r] : rb[r + 1]] = r // NCHUNK
        run_first[rb[r]] = True
        run_last[rb[r + 1] - 1] = True
    if pad_tail:
        cq[rb[-1] :] = NCHUNK - 1
        qtq[rb[-1] :] = QT - 1
        run_first[rb[-1]] = True
        run_last[S - 1] = True

    src_a = np.zeros((NC_, S * 128), np.int32)
    dstf_a = np.full((NC_, S * 128), -1.0, np.float32)
    ea_a = np.zeros((NC_, S * 128, K), np.float16)
    cum = np.zeros(NC_ * QT * NCHUNK + 1, np.int64)
    np.cumsum(counts.ravel(), out=cum[1:])
    for c in range(NC_):
        for r in range(QT * NCHUNK):
            k0 = cum[c * QT * NCHUNK + r]
            n_e = counts[c, r]
            sl = order[k0 : k0 + n_e]
            pos = rb[r] * 128
            src_a[c, pos : pos + n_e] = src[sl] - (r // NCHUNK) * QROWS
            dstf_a[c, pos : pos + n_e] = (dstl[sl] - (r % NCHUNK) * CHUNK
                                          ).astype(np.float32)
            ea_a[c, pos : pos + n_e] = ea[sl].astype(np.float16)

    src_d = src_a.reshape(NC_, S, 128).transpose(0, 2, 1).copy()
    dstf_d = dstf_a.reshape(NC_, S, 128).transpose(0, 2, 1).copy()
    eaT = ea_a.reshape(NC_, G, 512, K).transpose(0, 1, 3, 2)  # [NC, G, K, 512]
    Gp = (G + 1) // 2
    ea_d = np.zeros((NC_, 128, Gp * 512), np.float16)
    ev = eaT[:, 0::2]
    ea_d[:, :K, : ev.shape[1] * 512] = ev.transpose(0, 2, 1, 3).reshape(NC_, K, -1)
    od = eaT[:, 1::2]
    ea_d[:, K : 2 * K, : od.shape[1] * 512] = od.transpose(0, 2, 1, 3).reshape(NC_, K, -1)

    meta = dict(S=S, G=G, Gp=Gp,
                chunk_of_q=cq.tolist(), qt_of_q=qtq.tolist(),
                firsts=run_first.tolist(), lasts=run_last.tolist())
    return src_d, dstf_d, ea_d, meta


def _build(nc, meta):
    import contextlib
    import concourse.bass as bass
    import concourse.mybir as mybir
    import concourse.tile as tile

    F32, F16, BF16, I32 = (mybir.dt.float32, mybir.dt.float16,
                           mybir.dt.bfloat16, mybir.dt.int32)
    AF, ALU = mybir.ActivationFunctionType, mybir.AluOpType
    S, G, Gp = meta["S"], meta["G"], meta["Gp"]
    cq, firsts, lasts = meta["chunk_of_q"], meta["firsts"], meta["lasts"]
    qtq = meta["qt_of_q"]

    xT = nc.dram_tensor("xT", [F, NPAD], F16, kind="ExternalInput").ap()
    xTo = nc.dram_tensor("xTo", [F, NSHP], F16, kind="ExternalInput").ap()
    wpackb = nc.dram_tensor("wpackb", [F, 9 * F], BF16, kind="ExternalInput").ap()
    bpack = nc.dram_tensor("bpack", [F, 16], F32, kind="ExternalInput").ap()
    gw2 = nc.dram_tensor("gw2", [128, 128], F16, kind="ExternalInput").ap()
    wdiffb_in = nc.dram_tensor("wdiffb_in", [F, F], BF16, kind="ExternalInput").ap()
    iota_in = nc.dram_tensor("iota_in", [128, CHUNK], F16, kind="ExternalInput").ap()
    src_in = nc.dram_tensor("src_in", [128, S], I32, kind="ExternalInput").ap()
    dstf_in = nc.dram_tensor("dstf_in", [128, S], F32, kind="ExternalInput").ap()
    ea_in = nc.dram_tensor("ea_in", [128, Gp * 512], F16, kind="ExternalInput").ap()
    out0 = nc.dram_tensor("out0", [128, NBLK * 128], F16, kind="ExternalOutput").ap()
    out1 = nc.dram_tensor("out1", [128, NBLK * 128], F16, kind="ExternalOutput").ap()

    with tile.TileContext(nc) as tc, contextlib.ExitStack() as ctx:
        const = ctx.enter_context(tc.tile_pool(name="const", bufs=1))
        big = ctx.enter_context(tc.tile_pool(name="big", bufs=1))

        spEq = [nc.dram_tensor(f"spEq{k}", [QROWS, F], F16, kind="Internal").ap()
                for k in range(QT)]
        spEq_b = [t.rearrange("(b p) f -> p b f", p=128) for t in spEq]

        wpb = const.tile([F, 9 * F], BF16)
        nc.sync.dma_start(wpb[:], wpackb)
        bp = const.tile([F, 16], F32)
        nc.sync.dma_start(bp[:], bpack)
        gw = const.tile([128, 128], F16)
        nc.sync.dma_start(gw[:], gw2)
        wdb = const.tile([F, F], BF16)
        nc.sync.dma_start(wdb[:], wdiffb_in)
        iota = const.tile([128, CHUNK], F16)
        nc.sync.dma_start(iota[:], iota_in)
        srcs = const.tile([128, S], I32)
        nc.sync.dma_start(srcs[:], src_in)
        dstf = const.tile([128, S], F32)
        nc.sync.dma_start(dstf[:], dstf_in)
        half = const.tile([128, 1], F32)
        nc.gpsimd.memset(half[:], 0.5)

        W_sameT = wpb[:, F : 2 * F]
        W1T = [wpb[:, (2 + i) * F : (3 + i) * F] for i in range(3)]
        W2T = [wpb[:, (5 + i) * F : (6 + i) * F] for i in range(3)]
        W_lastT = wpb[:, 8 * F : 9 * F]
        b_diff = bp[:, 0:1]
        b_same = bp[:, 1:2]
        b1 = [bp[:, 2 + i : 3 + i] for i in range(3)]
        b2 = [bp[:, 5 + i : 6 + i] for i in range(3)]
        b_last = bp[:, 8:9]
        uT = bp[:, 9:10]

        z_sT = big.tile([128, NSHP], F32)
        xuT = big.tile([128, NSHP], F16)
        nc.gpsimd.memset(z_sT[:, NSH:NSHP], 0.0)

        # ---- table + phase 1b + edges, one scope, interleaved emission ----
        with tc.tile_pool(name="pxt", bufs=3) as pxt, \
             tc.tile_pool(name="pex", bufs=2) as pex, \
             tc.tile_pool(name="pxa", bufs=3) as pxa, \
             tc.tile_pool(name="pye", bufs=3) as pye, \
             tc.tile_pool(name="psp", bufs=3) as psp, \
             tc.tile_pool(name="prow", bufs=2) as prow, \
             tc.tile_pool(name="p1b", bufs=1) as p1b, \
             tc.tile_pool(name="pyg", bufs=7) as pyg, \
             tc.tile_pool(name="peat", bufs=3) as peat, \
             tc.tile_pool(name="pmsg", bufs=3) as pmsg, \
             tc.tile_pool(name="pgsb", bufs=6) as pgsb, \
             tc.tile_pool(name="poh", bufs=7) as poh, \
             tc.tile_pool(name="psA", bufs=2, space="PSUM") as psA, \
             tc.tile_pool(name="psG", bufs=2, space="PSUM") as psG, \
             tc.tile_pool(name="ps3", bufs=2, space="PSUM") as ps3, \
             tc.tile_pool(name="psAg", bufs=2, space="PSUM") as psAg:

            # phase 1b first (independent of table; z_sT = ssp(z_same))
            HW_ = NSHP // 2
            for hh in range(2):
                o = hh * HW_
                xto = p1b.tile([128, HW_], F16, tag="xto", name="xto")
                nc.sync.dma_start(xto[:], xTo[:, o : o + HW_])
                nc.vector.tensor_scalar_mul(xuT[:, o : o + HW_], xto[:], uT)
                ext = p1b.tile([128, HW_], F16, tag="ext", name="ext")
                nc.scalar.activation(ext[:], xto[:], AF.Exp)
                xa1 = p1b.tile([128, HW_], BF16, tag="xa1", name="xa1")
                nc.scalar.activation(xa1[:], ext[:], AF.Ln, bias=half[:, 0:1],
                                     scale=0.5)
                ez = p1b.tile([128, HW_], BF16, tag="ez", name="ez")
                for j in range(HW_ // 416):
                    ps = psA.tile([128, 512], F32, tag="mm")
                    nc.tensor.matmul(ps[:, :416], W_sameT,
                                     xa1[:, 416 * j : 416 * (j + 1)],
                                     start=True, stop=True, skip_group_check=True)
                    nc.scalar.activation(ez[:, 416 * j : 416 * (j + 1)],
                                         ps[:, :416], AF.Exp, bias=b_same)
                nc.scalar.activation(z_sT[:, o : o + HW_], ez[:], AF.Ln,
                                     bias=half[:, 0:1], scale=0.5)

            # phase 2 emission machinery
            oh_tiles = {}

            def build_oh(g):
                oh = poh.tile([128, 4, CHUNK], F16, tag="oh", name="oh")
                q0 = 4 * g
                for t in range(4):
                    nc.vector.tensor_scalar(oh[:, t, :], iota[:],
                                            dstf[:, q0 + t : q0 + t + 1], 0.0,
                                            ALU.subtract, ALU.is_equal)
                oh_tiles[g] = oh

            for g in range(min(2, G)):
                build_oh(g)

            st2 = dict(aggr=None, eat=None)
            scat_q = []
            SCAT_DELAY = 4

            def emit_supertile(g):
                q0 = 4 * g
                yg = pyg.tile([128, 4, F], F16, tag="yg", name="yg")
                for t in range(4):
                    q = q0 + t
                    nc.gpsimd.indirect_dma_start(
                        out=yg[:, t, :], out_offset=None,
                        in_=spEq[qtq[q]],
                        in_offset=bass.IndirectOffsetOnAxis(
                            ap=srcs[:, q : q + 1], axis=0))
                if g % 2 == 0:
                    eat = peat.tile([128, 512], F16, tag="eat", name="eat")
                    nc.sync.dma_start(eat[:], ea_in[:, 256 * g : 256 * g + 512])
                    st2["eat"] = eat
                eat = st2["eat"]
                rb_ = 64 * (g % 2)
                gps = psG.tile([128, 4, F], F32, tag="gate", name="gps")
                for t in range(4):
                    nc.tensor.matmul(gps[:, t, :],
                                     eat[rb_ : rb_ + 64, 128 * t : 128 * (t + 1)],
                                     gw[rb_ : rb_ + 64, :], start=True, stop=True,
                                     skip_group_check=True)
                gsb = pgsb.tile([128, 4, F], F16, tag="gsb", name="gsb")
                nc.vector.tensor_copy(gsb[:].rearrange("p a b -> p (a b)"),
                                      gps[:].rearrange("p a b -> p (a b)"))
                if g + 2 < G:
                    build_oh(g + 2)
                oh = oh_tiles.pop(g)
                scat_q.append((g, yg, gsb, oh))
                if len(scat_q) > SCAT_DELAY:
                    flush_scatter()

            def flush_scatter():
                g, yg, gsb, oh = scat_q.pop(0)
                q0 = 4 * g
                msg = pmsg.tile([128, 4, F], F16, tag="msg", name="msg")
                nc.vector.tensor_tensor(
                    msg[:].rearrange("p a b -> p (a b)"),
                    yg[:].rearrange("p a b -> p (a b)"),
                    gsb[:].rearrange("p a b -> p (a b)"), ALU.mult)
                for t in range(4):
                    q = q0 + t
                    c = cq[q]
                    if firsts[q]:
                        st2["aggr"] = psAg.tile([128, CHUNK], F32, tag="aggr",
                                                name="aggr")
                    nc.tensor.matmul(st2["aggr"][:], msg[:, t, :],
                                     oh[:, t, :], start=bool(firsts[q]),
                                     stop=bool(lasts[q]), skip_group_check=True)
                    if lasts[q]:
                        s0 = c * CHUNK
                        ncc = min(CHUNK, NSH - s0)
                        nc.vector.tensor_tensor(z_sT[:, s0 : s0 + ncc],
                                                z_sT[:, s0 : s0 + ncc],
                                                st2["aggr"][:, :ncc], ALU.add)

            # phase 3 emitter (reuses p1b pool tags; four chunk-aligned parts)
            P3_OFF = [0, 1536, 3072, 4608]      # col offsets (6,6,6,7 chunks)
            P3_W = [1536, 1536, 1536, 2048]
            P3_BO = [0, 12, 24, 36]             # block offsets
            P3_NB = [12, 12, 12, 13]
            P3_CHK = [(0, 6), (6, 12), (12, 18), (18, 25)]
            JW3 = 512

            def emit_phase3_part(p):
                o, HPW = P3_OFF[p], P3_W[p]
                nblk_h = P3_NB[p]
                wcols = nblk_h * 128
                cur = z_sT[:, o : o + HPW]
                xuh = xuT[:, o : o + HPW]
                o1c = p1b.tile([128, HPW], F16, tag="xto", name="o1c")
                nc.vector.tensor_copy(o1c[:, :wcols], cur[:, :wcols])
                o1r = p1b.tile([128, nblk_h, 128], F16, tag="ext", name="o1r")
                nc.sync.dma_start_transpose(o1r[:], o1c[:, :wcols])
                nc.sync.dma_start(
                    out1[:, P3_BO[p] * 128 : P3_BO[p] * 128 + wcols],
                    o1r[:].rearrange("p a b -> p (a b)"))
                for i in range(R):
                    e1 = p1b.tile([128, HPW], BF16, tag="xa1", name="e1")
                    nc.scalar.activation(e1[:], cur, AF.Exp)
                    s1 = p1b.tile([128, HPW], BF16, tag="ez", name="s1")
                    nc.scalar.activation(s1[:], e1[:], AF.Ln, bias=half[:, 0:1],
                                         scale=0.5)
                    e2 = p1b.tile([128, HPW], BF16, tag="xa1", name="e2")
                    for j in range(HPW // JW3):
                        ps = ps3.tile([128, JW3], F32, tag="mm3")
                        nc.tensor.matmul(ps[:], W1T[i],
                                         s1[:, JW3 * j : JW3 * (j + 1)],
                                         start=True, stop=True,
                                         skip_group_check=True)
                        nc.scalar.activation(e2[:, JW3 * j : JW3 * (j + 1)], ps[:],
                                             AF.Exp, bias=b1[i])
                    s2 = p1b.tile([128, HPW], BF16, tag="ez", name="s2")
                    nc.scalar.activation(s2[:], e2[:], AF.Ln, bias=half[:, 0:1],
                                         scale=0.5)
                    for j in range(HPW // JW3):
                        ps = ps3.tile([128, JW3], F32, tag="mm3")
                        nc.tensor.matmul(ps[:], W2T[i],
                                         s2[:, JW3 * j : JW3 * (j + 1)],
                                         start=True, stop=True,
                                         skip_group_check=True)
                        nc.vector.scalar_tensor_tensor(
                            cur[:, JW3 * j : JW3 * (j + 1)], ps[:], b2[i],
                            cur[:, JW3 * j : JW3 * (j + 1)], ALU.add, ALU.add)
                ev = p1b.tile([128, HPW], BF16, tag="xa1", name="ev")
                nc.scalar.activation(ev[:], cur, AF.Exp)
                sv = p1b.tile([128, HPW], BF16, tag="ez", name="sv")
                nc.scalar.activation(sv[:], ev[:], AF.Ln, bias=half[:, 0:1],
                                     scale=0.5)
                o0t = p1b.tile([128, HPW], F16, tag="xto", name="o0t")
                for j in range(HPW // JW3):
                    ps = ps3.tile([128, JW3], F32, tag="mm3")
                    nc.tensor.matmul(ps[:], W_lastT,
                                     sv[:, JW3 * j : JW3 * (j + 1)],
                                     start=True, stop=True, skip_group_check=True)
                    nc.vector.scalar_tensor_tensor(
                        o0t[:, JW3 * j : JW3 * (j + 1)], ps[:], b_last,
                        xuh[:, JW3 * j : JW3 * (j + 1)], ALU.add, ALU.add)
                o0r = p1b.tile([128, nblk_h, 128], F16, tag="ext", name="o0r")
                nc.sync.dma_start_transpose(o0r[:], o0t[:, :wcols])
                nc.sync.dma_start(
                    out0[:, P3_BO[p] * 128 : P3_BO[p] * 128 + wcols],
                    o0r[:].rearrange("p a b -> p (a b)"))

            # supertile after which all runs of each part's chunks closed
            g_part = [0, 0, 0, 0]
            for q in range(S):
                if lasts[q]:
                    for p, (c0, c1) in enumerate(P3_CHK):
                        if c0 <= cq[q] < c1:
                            g_part[p] = max(g_part[p], q // 4)
            p3_next = [0]

            def maybe_emit_p3():
                while (p3_next[0] < 4 and g_emit > g_part[p3_next[0]]):
                    while scat_q:
                        flush_scatter()
                    emit_phase3_part(p3_next[0])
                    p3_next[0] += 1

            # supertile g gatherable once quarter max(qtq[4g..4g+3]) is written
            g_need = [max(qtq[4 * g : 4 * g + 4]) for g in range(G)]
            g_emit = 0
            vt = 0.0
            vt_start = None

            # table build interleaved with phase-2 emission
            for g0 in range(0, NWA, GRPA):
                grp = range(g0, min(g0 + GRPA, NWA))
                xts, exs, xas, yes, sps, rows = {}, {}, {}, {}, {}, {}
                for w in grp:
                    xts[w] = pxt.tile([128, WINA], F16, tag="xt", name="xt")
                    nc.sync.dma_start(xts[w][:], xT[:, w * WINA : (w + 1) * WINA])
                for w in grp:
                    exs[w] = pex.tile([128, WINA], F16, tag="ex", name="ex")
                    nc.scalar.activation(exs[w][:], xts[w][:], AF.Exp)
                for w in grp:
                    xas[w] = pxa.tile([128, WINA], BF16, tag="xa", name="xa")
                    nc.scalar.activation(xas[w][:], exs[w][:], AF.Ln,
                                         bias=half[:, 0:1], scale=0.5)
                for w in grp:
                    yes[w] = pye.tile([128, WINA], F16, tag="ye", name="ye")
                    for j in range(4):
                        jw = 512 if j < 3 else 128
                        jo = 512 * j
                        ps = psA.tile([128, 512], F32, tag="mm")
                        nc.tensor.matmul(ps[:, :jw], wdb[:],
                                         xas[w][:, jo : jo + jw],
                                         start=True, stop=True,
                                         skip_group_check=True)
                        nc.scalar.activation(yes[w][:, jo : jo + jw],
                                             ps[:, :jw], AF.Exp, bias=b_diff)
                for w in grp:
                    sps[w] = psp.tile([128, WINA], F16, tag="sp", name="sp")
                    nc.scalar.activation(sps[w][:], yes[w][:], AF.Ln,
                                         bias=half[:, 0:1], scale=0.5)
                for w in grp:
                    rows[w] = prow.tile([128, WINA // 128, 128], F16, tag="row",
                                        name="row")
                    nc.scalar.dma_start_transpose(rows[w][:], sps[w][:])
                nb_w = WINA // 128
                for w in grp:
                    k, wi = w // 8, w % 8
                    nc.scalar.dma_start(spEq_b[k][:, wi * nb_w : (wi + 1) * nb_w, :],
                                        rows[w][:])
                # interleave phase-2 supertiles that are gatherable and paced
                vt += 34.0
                quarters_done = (g0 + GRPA) // 8
                if quarters_done >= 1:
                    if vt_start is None:
                        vt_start = vt
                    paced = int((vt - vt_start) / 2.2) + 14
                    while (g_emit < G and g_need[g_emit] < quarters_done - 1
                           and g_emit < paced):
                        emit_supertile(g_emit)
                        g_emit += 1
                        maybe_emit_p3()
            while g_emit < G:
                emit_supertile(g_emit)
                g_emit += 1
                maybe_emit_p3()
            while scat_q:
                flush_scatter()
            maybe_emit_p3()
            while p3_next[0] < 4:
                emit_phase3_part(p3_next[0])
                p3_next[0] += 1

    return nc


def kernel(**inputs):
    import concourse.bacc as bacc
    from concourse import bass_utils
    import ml_dtypes

    x = np.asarray(inputs["x"], np.float32)
    src_d, dstf_d, ea_d, meta = _prep(x, inputs["edge_index"], inputs["edge_attr"])

    key = (meta["S"], meta["G"])
    if key not in _cache:
        nc = bacc.Bacc("TRN2", target_bir_lowering=False, debug=False,
                       enable_asserts=False, num_devices=NC_,
                       dynamic_dma_scratch_size=32768)
        _build(nc, meta)
        nc.compile()
        _cache[key] = nc
    nc = _cache[key]

    wpack = np.concatenate(
        [np.asarray(inputs[k], np.float32).T.copy() for k in ["W_diff", "W_same"]]
        + [np.asarray(inputs["res_W1"][i], np.float32).T.copy() for i in range(3)]
        + [np.asarray(inputs["res_W2"][i], np.float32).T.copy() for i in range(3)]
        + [np.asarray(inputs["W_last"], np.float32).T.copy()], axis=1)
    wpackb = wpack.astype(ml_dtypes.bfloat16)
    wdiffb = np.asarray(inputs["W_diff"], np.float32).T.copy().astype(ml_dtypes.bfloat16)
    bpack = np.zeros((F, 16), np.float32)
    bpack[:, 0] = np.asarray(inputs["b_diff"], np.float32)
    bpack[:, 1] = np.asarray(inputs["b_same"], np.float32)
    for i in range(3):
        bpack[:, 2 + i] = np.asarray(inputs["res_b1"][i], np.float32)
        bpack[:, 5 + i] = np.asarray(inputs["res_b2"][i], np.float32)
    bpack[:, 8] = np.asarray(inputs["b_last"], np.float32)
    bpack[:, 9] = np.asarray(inputs["u"], np.float32)[0]
    G_w = np.asarray(inputs["G_w"], np.float32)
    gw2 = np.zeros((128, 128), np.float16)
    gw2[:K] = G_w.T.astype(np.float16)
    gw2[64 : 64 + K] = G_w.T.astype(np.float16)
    iota = np.broadcast_to(np.arange(CHUNK, dtype=np.float16), (128, CHUNK)).copy()

    xT = np.zeros((F, NPAD), np.float16)
    xT[:, :N] = x.T
    in_maps = []
    for c in range(NC_):
        xTo = np.zeros((F, NSHP), np.float16)
        xTo[:, :NSH] = x.T[:, c * NSH : (c + 1) * NSH]
        in_maps.append(dict(
            xT=xT, xTo=xTo, wpackb=wpackb, bpack=bpack, gw2=gw2, iota_in=iota,
            wdiffb_in=wdiffb, src_in=src_d[c], dstf_in=dstf_d[c], ea_in=ea_d[c],
        ))
    res = bass_utils.run_bass_kernel_spmd(nc, in_maps, core_ids=list(range(NC_)))

    def unblk(a):
        return (a.reshape(128, NBLK, 128).transpose(1, 0, 2)
                 .reshape(NBLK * 128, F)[:NSH].astype(np.float32))

    o0 = np.concatenate([unblk(res.results[c]["out0"]) for c in range(NC_)], axis=0)
    o1 = np.concatenate([unblk(res.results[c]["out1"]) for c in range(NC_)], axis=0)
    return (o0, o1)
